# revision 1
# baseline (speedup 1.0000x reference)
"""Bass/Trainium2 kernel for nn_GroundingLoss (symmetric token-level InfoNCE).

Math (matches the jax reference exactly):
    sim[a,b,i,j] = sum_k x[a,i,k] * z[b,j,k]
    S[a,b]       = (1/J) * sum_j  [ sum_i softmax_i(sim[a,b,:,j]) * sim[a,b,:,j] ]
    loss         = mean( logsumexp_a(S) - diag + logsumexp_b(S) - diag )

Sharding: the batch axis of x (a) is split across the 8 cores; every core
computes S[a_local, :] against all of z.

Device layout per core (v2): partitions = (a_sub, i) per a-tile (4 a's x 32
i's = 128), free = (b, j) in chunks of 1024 (32 b's x 32 j's, 8 chunks).
Per (chunk, atile) the PE computes sim = xT_t.T @ zT_c into PSUM (bf16
inputs, fp32 accum), ACT computes e = exp(sim - SHIFT) -> SBUF bf16, DVE
computes es = e * sim -> SBUF bf16.  The i-reductions (num = sum_i es,
den = sum_i e) are block-diagonal ones-matmuls on the PE accumulating over
all 8 a-tiles into PSUM [32, 1024] (DVE tensor_reduce has no fast mode on
TRN2, so reductions live on the PE instead).  Per chunk the DVE finishes:
r = num * recip(den), jsum = sum_j r -> out [32, 256].  The host divides by
J, assembles S, and does the tiny [256,256] logsumexp epilogue (softmax
weights are shift-invariant, so no SHIFT correction is needed).
"""

import numpy as np

N, I, J, K = 256, 32, 32, 256
NCORES = 8
NL = N // NCORES          # 32 local a's per core
AF = NL * I               # 1024 xt cols per K-half (a, i)
BJ = N * J                # 8192 (b, j) pairs
BJC = 512                 # free elements per chunk (16 b's x 32 j's)
NCHUNK = BJ // BJC        # 16
NAT = NL // 4             # 8 a-tiles of (4 a's x 32 i's) = 128 partitions
SHIFT = 60.0              # exp shift: safe for |sim| up to ~130

_cached = None


def _build():
    import concourse.bacc as bacc
    import concourse.mybir as mybir
    import concourse.tile as tile

    f32 = mybir.dt.float32
    bf16 = mybir.dt.bfloat16
    AF_T = mybir.ActivationFunctionType
    AX = mybir.AxisListType

    nc = bacc.Bacc("TRN2", target_bir_lowering=False, debug=False)
    xt_d = nc.dram_tensor("xt", [128, 2 * AF], bf16, kind="ExternalInput").ap()
    zt_d = nc.dram_tensor("zt", [128, 2 * BJ], bf16, kind="ExternalInput").ap()
    on_d = nc.dram_tensor("ones", [128, NAT * NL], bf16, kind="ExternalInput").ap()
    out_d = nc.dram_tensor("out", [NL, 2 * BJ], f32, kind="ExternalOutput").ap()

    with tile.TileContext(nc) as tc:
        with (
            tc.tile_pool(name="const", bufs=1) as cpool,
            tc.tile_pool(name="psum", bufs=3, space="PSUM") as ppool,
            tc.tile_pool(name="nd", bufs=2, space="PSUM") as ndpool,
            tc.tile_pool(name="sb", bufs=6) as spool,
            tc.tile_pool(name="ob", bufs=1) as opool,
        ):
            bias_t = cpool.tile([128, 1], f32)
            nc.gpsimd.memset(bias_t[:], -SHIFT)
            xt = cpool.tile([128, 2 * AF], bf16)
            nc.sync.dma_start(xt[:], xt_d[:, :])
            ones = cpool.tile([128, NAT * NL], bf16)
            nc.sync.dma_start(ones[:], on_d[:, :])
            zt = cpool.tile([128, 2 * BJ], bf16)
            # split the 4MB z load so early chunks can start before the tail
            nq = 4
            for kc in range(2):
                for q in range(nq):
                    sl = slice(kc * BJ + q * (BJ // nq), kc * BJ + (q + 1) * (BJ // nq))
                    nc.sync.dma_start(zt[:, sl], zt_d[:, sl])

            ob = opool.tile([NL, 2 * BJ], f32)

            for c in range(NCHUNK):
                num_ps = ndpool.tile([NL, BJC], f32, tag="num")
                den_ps = ndpool.tile([NL, BJC], f32, tag="den")
                for t in range(NAT):
                    sim = ppool.tile([128, BJC], f32, tag="sim")
                    for kc in range(2):
                        lhsT = xt[:, kc * AF + t * 128 : kc * AF + (t + 1) * 128]
                        rhs = zt[:, kc * BJ + c * BJC : kc * BJ + (c + 1) * BJC]
                        nc.tensor.matmul(
                            sim[:], lhsT, rhs, start=(kc == 0), stop=(kc == 1)
                        )

                    e = spool.tile([128, BJC], bf16, tag="e")
                    nc.scalar.activation(e[:], sim[:], AF_T.Exp, bias=bias_t[:], scale=1.0)
                    es = spool.tile([128, BJC], bf16, tag="es")
                    nc.vector.tensor_mul(es[:], e[:], sim[:])

                    onesT = ones[:, t * NL : (t + 1) * NL]
                    nc.tensor.matmul(
                        num_ps[:], onesT, es[:],
                        start=(t == 0), stop=(t == NAT - 1),
                    )
                    nc.tensor.matmul(
                        den_ps[:], onesT, e[:],
                        start=(t == 0), stop=(t == NAT - 1),
                    )

                # ship num/den to the host (device division + j-sum stalls
                # the PE via the slow DVE reciprocal); cheap PSUM->SBUF copies
                nc.vector.tensor_copy(ob[:, 2 * c * BJC : 2 * c * BJC + BJC], num_ps[:])
                nc.scalar.activation(
                    ob[:, 2 * c * BJC + BJC : 2 * (c + 1) * BJC], den_ps[:], AF_T.Copy
                )
            nc.sync.dma_start(out_d[:, :], ob[:])
    nc.compile()
    return nc


def _prep_inputs(x, z):
    import ml_dtypes

    bf = ml_dtypes.bfloat16
    x = np.ascontiguousarray(x, dtype=np.float32).astype(bf)
    z = np.ascontiguousarray(z, dtype=np.float32).astype(bf)
    # zT[p, kc*BJ + b*J + j] = z[b, j, kc*128 + p]
    zt = z.transpose(2, 0, 1).reshape(K, BJ)
    zt = np.concatenate([zt[0:128], zt[128:256]], axis=1)
    zt = np.ascontiguousarray(zt)
    # block-diagonal ones: ones[p, t*NL + 4t + p//32] = 1
    on = np.zeros((128, NAT * NL), dtype=bf)
    for t in range(NAT):
        for p in range(128):
            on[p, t * NL + 4 * t + p // 32] = 1
    in_maps = []
    for d in range(NCORES):
        xl = x[d * NL : (d + 1) * NL]                  # [NL, I, K]
        xt = xl.transpose(2, 0, 1).reshape(K, AF)      # [K, (a,i)]
        xt = np.concatenate([xt[0:128], xt[128:256]], axis=1)
        in_maps.append({"xt": np.ascontiguousarray(xt), "zt": zt, "ones": on})
    return in_maps


def _epilogue(results):
    S = np.empty((N, N), dtype=np.float64)
    for d in range(NCORES):
        arr = results[d]["out"].astype(np.float64).reshape(NL, NCHUNK, 2, BJC)
        r = arr[:, :, 0, :] / arr[:, :, 1, :]          # [NL, chunk, (b,j)]
        r = r.reshape(NL, NCHUNK, BJC // J, J).mean(axis=3)
        S[d * NL : (d + 1) * NL, :] = r.reshape(NL, N)
    diag = np.diagonal(S)
    m0 = S.max(axis=0)
    lx = m0 + np.log(np.exp(S - m0[None, :]).sum(axis=0)) - diag
    m1 = S.max(axis=1)
    lz = m1 + np.log(np.exp(S - m1[:, None]).sum(axis=1)) - diag
    loss = (lx + lz).mean()
    return np.asarray(loss, dtype=np.float32)


def run_on_device(x, z, trace=False):
    """Returns (loss, BassKernelResults)."""
    from concourse.bass_utils import run_bass_kernel_spmd

    global _cached
    if _cached is None:
        _cached = _build()
    nc = _cached
    in_maps = _prep_inputs(x, z)
    res = run_bass_kernel_spmd(nc, in_maps, list(range(NCORES)), trace=trace)
    return _epilogue(res.results), res


def kernel(x, z):
    loss, _ = run_on_device(x, z)
    return loss



# revision 14
# speedup vs baseline: 1.6731x; 1.6731x over previous
"""Bass/Trainium2 kernel for nn_GroundingLoss (symmetric token-level InfoNCE).

Math (matches the jax reference exactly):
    sim[a,b,i,j] = sum_k x[a,i,k] * z[b,j,k]
    S[a,b]       = (1/J) * sum_j  [ sum_i softmax_i(sim[a,b,:,j]) * sim[a,b,:,j] ]
    loss         = mean( logsumexp_a(S) - diag + logsumexp_b(S) - diag )

Sharding: the batch axis of x (a) is split across the 8 cores; every core
computes S[a_local, :] against all of z.

v3 design (per core):
  partitions = (a_sub, i) per a-tile (4 a's x 32 i's = 128), free = (b, j).
  sim via fp8e4m3 DoubleRow matmuls: lhsT xt8 [128,(2,128)], rhs zt8
  [128,(2,512)] -> one matmul contracts all K=256 at 0.5 cyc/col (4x fewer
  PE cycles than the bf16 K-half pair; host-measured fp8 loss rel-err
  ~1.2e-3, well inside the 2e-2 gate).  ACT computes e = exp(sim - SHIFT)
  on chunk-PAIRS [128,1024] PSUM->SBUF bf16; DVE and Pool alternate the
  es = e * sim product.  The i-reductions stay on the PE as block-diagonal
  bf16 ones-matmuls: both chunks of a pair accumulate into one PSUM bank
  via output base partitions {0, 32} (num_p/den_p [64, 512], row =
  32*(c%2) + a_local).  Each pair's num/den is DMA'd PSUM->DRAM directly.
  Loop over pair-blocks with the ones-matmuls skewed one block behind the
  sim matmuls so the PE never waits on the exp/mul chain and stays in the
  high p-state.  The host does the tiny division + j-sum + [256,256]
  logsumexp epilogue.
"""

import numpy as np

N, I, J, K = 256, 32, 32, 256
NCORES = 8
NL = N // NCORES          # 32 local a's per core
AF = NL * I               # 1024 xt cols (a, i)
BJ = N * J                # 8192 (b, j) pairs
BJC = 512                 # free elements per chunk (16 b's x 32 j's)
NCHUNK = BJ // BJC        # 16
NPAIR = NCHUNK // 2       # 8 chunk-pairs
NAT = NL // 4             # 8 a-tiles of (4 a's x 32 i's) = 128 partitions
SHIFT = 60.0              # exp shift: safe for |sim| up to ~130
COPY_EVERY = 16           # every COPY_EVERY-th pair: ACT copies sim to bf16
                          # SBUF so that pair's DVE mul runs in 2x_1p mode

_cached = None


def _build():
    import concourse.bacc as bacc
    import concourse.mybir as mybir
    import concourse.tile as tile

    f32 = mybir.dt.float32
    bf16 = mybir.dt.bfloat16
    fp8 = mybir.dt.float8e4
    AF_T = mybir.ActivationFunctionType
    DR = mybir.MatmulPerfMode.DoubleRow

    nc = bacc.Bacc("TRN2", target_bir_lowering=False, debug=False)
    xt_d = nc.dram_tensor("xt", [128, 2, AF], fp8, kind="ExternalInput").ap()
    zt_d = nc.dram_tensor("zt", [128, 2, BJ], fp8, kind="ExternalInput").ap()
    on_d = nc.dram_tensor("ones", [128, NAT * NL], bf16, kind="ExternalInput").ap()
    # out: per pair [64, num(512) | den(512)] f32, rows = 32*(c%2) + a_local
    out_d = nc.dram_tensor("out", [64, NPAIR, 2, BJC], f32, kind="ExternalOutput").ap()

    with tile.TileContext(nc) as tc:
        with (
            tc.tile_pool(name="const", bufs=1) as cpool,
            tc.tile_pool(name="simp", bufs=3, space="PSUM") as ppool,
            tc.tile_pool(name="nd", bufs=1, space="PSUM") as ndpool,
            tc.tile_pool(name="combo", bufs=10) as copool,
            tc.tile_pool(name="simbf", bufs=2) as sbfpool,
            tc.tile_pool(name="ndsb", bufs=2) as ndsbpool,
        ):
            bias_t = cpool.tile([128, 1], f32)
            nc.gpsimd.memset(bias_t[:], -SHIFT)
            xt = cpool.tile([128, 2, AF], fp8)
            nc.sync.dma_start(xt[:], xt_d[:, :, :])
            ones = cpool.tile([128, NAT * NL], bf16)
            nc.sync.dma_start(ones[:], on_d[:, :])
            zt = cpool.tile([128, 2, BJ], fp8)
            # split the z load so the first pair-block can start before the tail
            for qd in range(4):
                sl = slice(qd * (BJ // 4), (qd + 1) * (BJ // 4))
                nc.sync.dma_start(zt[:, :, sl], zt_d[:, :, sl])

            # combo tile layout per (pp, t): [es(c0) es(c1) e(c0) e(c1)], bf16
            combos = {}
            nmul = 0
            for pp in range(NPAIR + 1):
                for t in range(NAT):
                    if pp <= NPAIR - 1:
                        # sim for pair pp, atile t: [128, (2 chunks x 512)]
                        lhsT = xt[:, :, t * 128 : (t + 1) * 128]
                        sim = ppool.tile([128, 2, BJC], f32, tag="sim")
                        for h in range(2):
                            c = 2 * pp + h
                            nc.tensor.matmul(
                                sim[:, h, :], lhsT,
                                zt[:, :, c * BJC : (c + 1) * BJC],
                                start=True, stop=True, perf_mode=DR,
                            )
                        co = copool.tile([128, 4, BJC], bf16, tag="combo")
                        # e pair: exp(sim - SHIFT), one ACT instr [128,1024]
                        nc.scalar.activation(
                            co[:, 2:4, :], sim[:, :, :], AF_T.Exp,
                            bias=bias_t[:], scale=1.0,
                        )
                        # es pair: e * sim, one DVE instr; every COPY_EVERY-th
                        # pair goes through an ACT bf16 copy so the DVE mul
                        # gets the 2x all-16-bit fast path
                        if nmul % COPY_EVERY == COPY_EVERY - 1:
                            sbf = sbfpool.tile([128, 2, BJC], bf16, tag="sbf")
                            nc.scalar.activation(sbf[:], sim[:, :, :], AF_T.Copy)
                            nc.vector.tensor_mul(co[:, 0:2, :], co[:, 2:4, :], sbf[:])
                        else:
                            nc.vector.tensor_mul(co[:, 0:2, :], co[:, 2:4, :], sim[:, :, :])
                        nmul += 1
                        combos[(pp, t)] = co
                    if pp > 0:
                        # reductions for pair pp-1, atile t -> stacked PSUM rows
                        onesT = ones[:, t * NL : (t + 1) * NL]
                        if t == 0:
                            nd = ndpool.tile([64, 2, BJC], f32, tag="nd")
                        co = combos[(pp - 1, t)]
                        st, sp = (t == 0), (t == NAT - 1)
                        for q in range(2):
                            nc.tensor.matmul(
                                nd[32 * q : 32 * (q + 1), 0, :],
                                onesT, co[:, q, :],
                                start=st, stop=sp,
                            )
                            nc.tensor.matmul(
                                nd[32 * q : 32 * (q + 1), 1, :],
                                onesT, co[:, 2 + q, :],
                                start=st, stop=sp,
                            )
                if pp > 0:
                    # stage num|den to SBUF (DMA cannot read PSUM), then ship
                    ndsb = ndsbpool.tile([64, 2, BJC], f32, tag="ndsb")
                    nc.scalar.activation(ndsb[:], nd[:], AF_T.Copy)
                    nc.sync.dma_start(out_d[:, pp - 1, :, :], ndsb[:])
    nc.compile()
    return nc


def _prep_inputs(x, z):
    import ml_dtypes

    f8 = ml_dtypes.float8_e4m3fn
    x = np.ascontiguousarray(x, dtype=np.float32).astype(f8)
    z = np.ascontiguousarray(z, dtype=np.float32).astype(f8)
    # zt[p, kc, b*J + j] = z[b, j, kc*128 + p]
    zt = z.transpose(2, 0, 1).reshape(K, BJ)
    zt = np.ascontiguousarray(np.stack([zt[0:128], zt[128:256]], axis=1))
    # block-diagonal ones: tile t's lhsT [128, 32] has its 1 at column
    # 4t + p//32, so output row = a_local for the 4 a's the tile covers
    on = np.zeros((128, NAT * NL), dtype=ml_dtypes.bfloat16)
    for t in range(NAT):
        for p in range(128):
            on[p, t * NL + 4 * t + p // 32] = 1
    in_maps = []
    for d in range(NCORES):
        xl = x[d * NL : (d + 1) * NL]                  # [NL, I, K]
        xt = xl.transpose(2, 0, 1).reshape(K, AF)      # [K, (a,i)]
        xt = np.ascontiguousarray(np.stack([xt[0:128], xt[128:256]], axis=1))
        in_maps.append({"xt": xt, "zt": zt, "ones": on})
    return in_maps


def _epilogue(results):
    S = np.empty((N, N), dtype=np.float64)
    for d in range(NCORES):
        arr = results[d]["out"].astype(np.float64)     # [64, NPAIR, 2, BJC]
        r = arr[:, :, 0, :] / arr[:, :, 1, :]          # [64, pair, 512]
        # row p = 32*q + a ; chunk c = 2*pp + q ; col = (b - 16c)*32 + j
        r = r.reshape(2, NL, NPAIR, BJC // J, J).mean(axis=4)  # [q, a, pp, 16]
        for q in range(2):
            for pp in range(NPAIR):
                c = 2 * pp + q
                S[d * NL : (d + 1) * NL, 16 * c : 16 * (c + 1)] = r[q, :, pp, :]
    diag = np.diagonal(S)
    m0 = S.max(axis=0)
    lx = m0 + np.log(np.exp(S - m0[None, :]).sum(axis=0)) - diag
    m1 = S.max(axis=1)
    lz = m1 + np.log(np.exp(S - m1[:, None]).sum(axis=1)) - diag
    loss = (lx + lz).mean()
    return np.asarray(loss, dtype=np.float32)


def run_on_device(x, z, trace=False):
    """Returns (loss, BassKernelResults)."""
    from concourse.bass_utils import run_bass_kernel_spmd

    global _cached
    if _cached is None:
        _cached = _build()
    nc = _cached
    in_maps = _prep_inputs(x, z)
    res = run_bass_kernel_spmd(nc, in_maps, list(range(NCORES)), trace=trace)
    return _epilogue(res.results), res


def kernel(x, z):
    loss, _ = run_on_device(x, z)
    return loss


# revision 17
# speedup vs baseline: 1.7041x; 1.0185x over previous
"""Bass/Trainium2 kernel for nn_GroundingLoss (symmetric token-level InfoNCE).

Math (matches the jax reference exactly):
    sim[a,b,i,j] = sum_k x[a,i,k] * z[b,j,k]
    S[a,b]       = (1/J) * sum_j  [ sum_i softmax_i(sim[a,b,:,j]) * sim[a,b,:,j] ]
    loss         = mean( logsumexp_a(S) - diag + logsumexp_b(S) - diag )

Sharding: the batch axis of x (a) is split across the 8 cores; every core
computes S[a_local, :] against all of z.

v3 design (per core):
  partitions = (a_sub, i) per a-tile (4 a's x 32 i's = 128), free = (b, j).
  sim via fp8e4m3 DoubleRow matmuls: lhsT xt8 [128,(2,128)], rhs zt8
  [128,(2,512)] -> one matmul contracts all K=256 at 0.5 cyc/col (4x fewer
  PE cycles than the bf16 K-half pair; host-measured fp8 loss rel-err
  ~1.2e-3, well inside the 2e-2 gate).  ACT computes e = exp(sim - SHIFT)
  on chunk-PAIRS [128,1024] PSUM->SBUF bf16; DVE and Pool alternate the
  es = e * sim product.  The i-reductions stay on the PE as block-diagonal
  bf16 ones-matmuls: both chunks of a pair accumulate into one PSUM bank
  via output base partitions {0, 32} (num_p/den_p [64, 512], row =
  32*(c%2) + a_local).  Each pair's num/den is DMA'd PSUM->DRAM directly.
  Loop over pair-blocks with the ones-matmuls skewed one block behind the
  sim matmuls so the PE never waits on the exp/mul chain and stays in the
  high p-state.  The host does the tiny division + j-sum + [256,256]
  logsumexp epilogue.
"""

import numpy as np

N, I, J, K = 256, 32, 32, 256
NCORES = 8
NL = N // NCORES          # 32 local a's per core
AF = NL * I               # 1024 xt cols (a, i)
BJ = N * J                # 8192 (b, j) pairs
BJC = 512                 # free elements per chunk (16 b's x 32 j's)
NCHUNK = BJ // BJC        # 16
NPAIR = NCHUNK // 2       # 8 chunk-pairs
NAT = NL // 4             # 8 a-tiles of (4 a's x 32 i's) = 128 partitions
SHIFT = 60.0              # exp shift: safe for |sim| up to ~130
SKEW = 4                  # ones-matmuls trail the sim matmuls by SKEW steps

_cached = None


def _build():
    import concourse.bacc as bacc
    import concourse.mybir as mybir
    import concourse.tile as tile

    f32 = mybir.dt.float32
    bf16 = mybir.dt.bfloat16
    fp8 = mybir.dt.float8e4
    AF_T = mybir.ActivationFunctionType
    DR = mybir.MatmulPerfMode.DoubleRow

    nc = bacc.Bacc("TRN2", target_bir_lowering=False, debug=False)
    xt_d = nc.dram_tensor("xt", [128, 2, AF], fp8, kind="ExternalInput").ap()
    zt_d = nc.dram_tensor("zt", [128, 2, BJ], fp8, kind="ExternalInput").ap()
    on_d = nc.dram_tensor("ones", [128, NAT * NL], bf16, kind="ExternalInput").ap()
    # out: per pair [64, num(512) | den(512)] f32, rows = 32*(c%2) + a_local
    out_d = nc.dram_tensor("out", [64, NPAIR, 2, BJC], f32, kind="ExternalOutput").ap()

    with tile.TileContext(nc) as tc:
        with (
            tc.tile_pool(name="const", bufs=1) as cpool,
            tc.tile_pool(name="simp", bufs=3, space="PSUM") as ppool,
            tc.tile_pool(name="nd", bufs=1, space="PSUM") as ndpool,
            tc.tile_pool(name="combo", bufs=10) as copool,
            tc.tile_pool(name="ndsb", bufs=2) as ndsbpool,
        ):
            bias_t = cpool.tile([128, 1], f32)
            nc.gpsimd.memset(bias_t[:], -SHIFT)
            xt = cpool.tile([128, 2, AF], fp8)
            nc.sync.dma_start(xt[:], xt_d[:, :, :])
            ones = cpool.tile([128, NAT * NL], bf16)
            nc.sync.dma_start(ones[:], on_d[:, :])
            zt = cpool.tile([128, 2, BJ], fp8)
            # split the z load so the first pair-block can start before the tail
            for qd in range(4):
                sl = slice(qd * (BJ // 4), (qd + 1) * (BJ // 4))
                nc.sync.dma_start(zt[:, :, sl], zt_d[:, :, sl])

            # combo tile layout per (pp, t): [es(c0) es(c1) e(c0) e(c1)], bf16
            combos = {}
            nd = None
            NSTEP = NPAIR * NAT
            for u in range(NSTEP + SKEW):
                if u < NSTEP:
                    pp, t = divmod(u, NAT)
                    # sim for pair pp, atile t: [128, (2 chunks x 512)]
                    lhsT = xt[:, :, t * 128 : (t + 1) * 128]
                    sim = ppool.tile([128, 2, BJC], f32, tag="sim")
                    for h in range(2):
                        c = 2 * pp + h
                        nc.tensor.matmul(
                            sim[:, h, :], lhsT,
                            zt[:, :, c * BJC : (c + 1) * BJC],
                            start=True, stop=True, perf_mode=DR,
                        )
                    co = copool.tile([128, 4, BJC], bf16, tag="combo")
                    # e pair: exp(sim - SHIFT), one ACT instr [128,1024]
                    nc.scalar.activation(
                        co[:, 2:4, :], sim[:, :, :], AF_T.Exp,
                        bias=bias_t[:], scale=1.0,
                    )
                    # es pair: e * sim, one DVE instr [128,1024]
                    nc.vector.tensor_mul(co[:, 0:2, :], co[:, 2:4, :], sim[:, :, :])
                    combos[(pp, t)] = co
                v = u - SKEW
                if v >= 0:
                    pq, tq = divmod(v, NAT)
                    # reductions for pair pq, atile tq -> stacked PSUM rows
                    onesT = ones[:, tq * NL : (tq + 1) * NL]
                    if tq == 0:
                        nd = ndpool.tile([64, 2, BJC], f32, tag="nd")
                    co = combos.pop((pq, tq))
                    st, sp = (tq == 0), (tq == NAT - 1)
                    for q in range(2):
                        nc.tensor.matmul(
                            nd[32 * q : 32 * (q + 1), 0, :],
                            onesT, co[:, q, :],
                            start=st, stop=sp,
                        )
                        nc.tensor.matmul(
                            nd[32 * q : 32 * (q + 1), 1, :],
                            onesT, co[:, 2 + q, :],
                            start=st, stop=sp,
                        )
                    if tq == NAT - 1:
                        # stage num|den to SBUF (DMA cannot read PSUM); the
                        # last block's copy runs on DVE to balance ACT/DVE
                        ndsb = ndsbpool.tile([64, 2, BJC], f32, tag="ndsb")
                        if pq == NPAIR - 1:
                            nc.vector.tensor_copy(ndsb[:], nd[:])
                        else:
                            nc.scalar.activation(ndsb[:], nd[:], AF_T.Copy)
                        nc.sync.dma_start(out_d[:, pq, :, :], ndsb[:])
    nc.compile()
    return nc


def _prep_inputs(x, z):
    import ml_dtypes

    f8 = ml_dtypes.float8_e4m3fn
    x = np.ascontiguousarray(x, dtype=np.float32).astype(f8)
    z = np.ascontiguousarray(z, dtype=np.float32).astype(f8)
    # zt[p, kc, b*J + j] = z[b, j, kc*128 + p]
    zt = z.transpose(2, 0, 1).reshape(K, BJ)
    zt = np.ascontiguousarray(np.stack([zt[0:128], zt[128:256]], axis=1))
    # block-diagonal ones: tile t's lhsT [128, 32] has its 1 at column
    # 4t + p//32, so output row = a_local for the 4 a's the tile covers
    on = np.zeros((128, NAT * NL), dtype=ml_dtypes.bfloat16)
    for t in range(NAT):
        for p in range(128):
            on[p, t * NL + 4 * t + p // 32] = 1
    in_maps = []
    for d in range(NCORES):
        xl = x[d * NL : (d + 1) * NL]                  # [NL, I, K]
        xt = xl.transpose(2, 0, 1).reshape(K, AF)      # [K, (a,i)]
        xt = np.ascontiguousarray(np.stack([xt[0:128], xt[128:256]], axis=1))
        in_maps.append({"xt": xt, "zt": zt, "ones": on})
    return in_maps


def _epilogue(results):
    S = np.empty((N, N), dtype=np.float64)
    for d in range(NCORES):
        arr = results[d]["out"].astype(np.float64)     # [64, NPAIR, 2, BJC]
        r = arr[:, :, 0, :] / arr[:, :, 1, :]          # [64, pair, 512]
        # row p = 32*q + a ; chunk c = 2*pp + q ; col = (b - 16c)*32 + j
        r = r.reshape(2, NL, NPAIR, BJC // J, J).mean(axis=4)  # [q, a, pp, 16]
        for q in range(2):
            for pp in range(NPAIR):
                c = 2 * pp + q
                S[d * NL : (d + 1) * NL, 16 * c : 16 * (c + 1)] = r[q, :, pp, :]
    diag = np.diagonal(S)
    m0 = S.max(axis=0)
    lx = m0 + np.log(np.exp(S - m0[None, :]).sum(axis=0)) - diag
    m1 = S.max(axis=1)
    lz = m1 + np.log(np.exp(S - m1[:, None]).sum(axis=1)) - diag
    loss = (lx + lz).mean()
    return np.asarray(loss, dtype=np.float32)


def run_on_device(x, z, trace=False):
    """Returns (loss, BassKernelResults)."""
    from concourse.bass_utils import run_bass_kernel_spmd

    global _cached
    if _cached is None:
        _cached = _build()
    nc = _cached
    in_maps = _prep_inputs(x, z)
    res = run_bass_kernel_spmd(nc, in_maps, list(range(NCORES)), trace=trace)
    return _epilogue(res.results), res


def kernel(x, z):
    loss, _ = run_on_device(x, z)
    return loss


# revision 20
# speedup vs baseline: 1.7108x; 1.0039x over previous
"""Bass/Trainium2 kernel for nn_GroundingLoss (symmetric token-level InfoNCE).

Math (matches the jax reference exactly):
    sim[a,b,i,j] = sum_k x[a,i,k] * z[b,j,k]
    S[a,b]       = (1/J) * sum_j  [ sum_i softmax_i(sim[a,b,:,j]) * sim[a,b,:,j] ]
    loss         = mean( logsumexp_a(S) - diag + logsumexp_b(S) - diag )

Sharding: the batch axis of x (a) is split across the 8 cores; every core
computes S[a_local, :] against all of z.

v3 design (per core):
  partitions = (a_sub, i) per a-tile (4 a's x 32 i's = 128), free = (b, j).
  sim via fp8e4m3 DoubleRow matmuls: lhsT xt8 [128,(2,128)], rhs zt8
  [128,(2,512)] -> one matmul contracts all K=256 at 0.5 cyc/col (4x fewer
  PE cycles than the bf16 K-half pair; host-measured fp8 loss rel-err
  ~1.2e-3, well inside the 2e-2 gate).  ACT computes e = exp(sim - SHIFT)
  on chunk-PAIRS [128,1024] PSUM->SBUF bf16; DVE and Pool alternate the
  es = e * sim product.  The i-reductions stay on the PE as block-diagonal
  bf16 ones-matmuls: both chunks of a pair accumulate into one PSUM bank
  via output base partitions {0, 32} (num_p/den_p [64, 512], row =
  32*(c%2) + a_local).  Each pair's num/den is DMA'd PSUM->DRAM directly.
  Loop over pair-blocks with the ones-matmuls skewed one block behind the
  sim matmuls so the PE never waits on the exp/mul chain and stays in the
  high p-state.  The host does the tiny division + j-sum + [256,256]
  logsumexp epilogue.
"""

import numpy as np

N, I, J, K = 256, 32, 32, 256
NCORES = 8
NL = N // NCORES          # 32 local a's per core
AF = NL * I               # 1024 xt cols (a, i)
BJ = N * J                # 8192 (b, j) pairs
BJC = 512                 # free elements per chunk (16 b's x 32 j's)
NCHUNK = BJ // BJC        # 16
NPAIR = NCHUNK // 2       # 8 chunk-pairs
NAT = NL // 4             # 8 a-tiles of (4 a's x 32 i's) = 128 partitions
SHIFT = 60.0              # exp shift: safe for |sim| up to ~130
SKEW = 3                  # ones-matmuls trail the sim matmuls by SKEW steps

_cached = None


def _build():
    import concourse.bacc as bacc
    import concourse.mybir as mybir
    import concourse.tile as tile

    f32 = mybir.dt.float32
    bf16 = mybir.dt.bfloat16
    fp8 = mybir.dt.float8e4
    AF_T = mybir.ActivationFunctionType
    DR = mybir.MatmulPerfMode.DoubleRow

    nc = bacc.Bacc("TRN2", target_bir_lowering=False, debug=False)
    xt_d = nc.dram_tensor("xt", [128, 2, AF], fp8, kind="ExternalInput").ap()
    zt_d = nc.dram_tensor("zt", [128, 2, BJ], fp8, kind="ExternalInput").ap()
    on_d = nc.dram_tensor("ones", [128, NAT * NL], bf16, kind="ExternalInput").ap()
    # out: per pair [64, num(512) | den(512)] f32, rows = 32*(c%2) + a_local
    out_d = nc.dram_tensor("out", [64, NPAIR, 2, BJC], f32, kind="ExternalOutput").ap()

    with tile.TileContext(nc) as tc:
        with (
            tc.tile_pool(name="const", bufs=1) as cpool,
            tc.tile_pool(name="simp", bufs=3, space="PSUM") as ppool,
            tc.tile_pool(name="nd", bufs=1, space="PSUM") as ndpool,
            tc.tile_pool(name="combo", bufs=10) as copool,
            tc.tile_pool(name="ndsb", bufs=2) as ndsbpool,
        ):
            bias_t = cpool.tile([128, 1], f32)
            nc.gpsimd.memset(bias_t[:], -SHIFT)
            xt = cpool.tile([128, 2, AF], fp8)
            nc.gpsimd.dma_start(xt[:], xt_d[:, :, :])
            ones = cpool.tile([128, NAT * NL], bf16)
            nc.sync.dma_start(ones[:], on_d[:, :])
            zt = cpool.tile([128, 2, BJ], fp8)
            # split the z load across four queues so the first pair-block's
            # slice lands early instead of waiting on one serialized queue
            for qd, eng in enumerate((nc.gpsimd, nc.scalar, nc.gpsimd, nc.sync)):
                sl = slice(qd * (BJ // 4), (qd + 1) * (BJ // 4))
                eng.dma_start(zt[:, :, sl], zt_d[:, :, sl])

            # combo tile layout per (pp, t): [es(c0) es(c1) e(c0) e(c1)], bf16
            combos = {}
            nd = None
            NSTEP = NPAIR * NAT
            for u in range(NSTEP + SKEW):
                if u < NSTEP:
                    pp, t = divmod(u, NAT)
                    # sim for pair pp, atile t: [128, (2 chunks x 512)]
                    lhsT = xt[:, :, t * 128 : (t + 1) * 128]
                    sim = ppool.tile([128, 2, BJC], f32, tag="sim")
                    for h in range(2):
                        c = 2 * pp + h
                        nc.tensor.matmul(
                            sim[:, h, :], lhsT,
                            zt[:, :, c * BJC : (c + 1) * BJC],
                            start=True, stop=True, perf_mode=DR,
                        )
                    co = copool.tile([128, 4, BJC], bf16, tag="combo")
                    # e pair: exp(sim - SHIFT), one ACT instr [128,1024]
                    nc.scalar.activation(
                        co[:, 2:4, :], sim[:, :, :], AF_T.Exp,
                        bias=bias_t[:], scale=1.0,
                    )
                    # es pair: e * sim, one DVE instr [128,1024]
                    nc.vector.tensor_mul(co[:, 0:2, :], co[:, 2:4, :], sim[:, :, :])
                    combos[(pp, t)] = co
                v = u - SKEW
                if v >= 0:
                    pq, tq = divmod(v, NAT)
                    # reductions for pair pq, atile tq -> stacked PSUM rows
                    onesT = ones[:, tq * NL : (tq + 1) * NL]
                    if tq == 0:
                        nd = ndpool.tile([64, 2, BJC], f32, tag="nd")
                    co = combos.pop((pq, tq))
                    st, sp = (tq == 0), (tq == NAT - 1)
                    for q in range(2):
                        nc.tensor.matmul(
                            nd[32 * q : 32 * (q + 1), 0, :],
                            onesT, co[:, q, :],
                            start=st, stop=sp,
                        )
                        nc.tensor.matmul(
                            nd[32 * q : 32 * (q + 1), 1, :],
                            onesT, co[:, 2 + q, :],
                            start=st, stop=sp,
                        )
                    if tq == NAT - 1:
                        # stage num|den to SBUF (DMA cannot read PSUM); the
                        # last block's copy runs on DVE to balance ACT/DVE
                        ndsb = ndsbpool.tile([64, 2, BJC], f32, tag="ndsb")
                        if pq == NPAIR - 1:
                            nc.vector.tensor_copy(ndsb[:], nd[:])
                        else:
                            nc.scalar.activation(ndsb[:], nd[:], AF_T.Copy)
                        nc.sync.dma_start(out_d[:, pq, :, :], ndsb[:])
    nc.compile()
    return nc


def _prep_inputs(x, z):
    import ml_dtypes

    f8 = ml_dtypes.float8_e4m3fn
    x = np.ascontiguousarray(x, dtype=np.float32).astype(f8)
    z = np.ascontiguousarray(z, dtype=np.float32).astype(f8)
    # zt[p, kc, b*J + j] = z[b, j, kc*128 + p]
    zt = z.transpose(2, 0, 1).reshape(K, BJ)
    zt = np.ascontiguousarray(np.stack([zt[0:128], zt[128:256]], axis=1))
    # block-diagonal ones: tile t's lhsT [128, 32] has its 1 at column
    # 4t + p//32, so output row = a_local for the 4 a's the tile covers
    on = np.zeros((128, NAT * NL), dtype=ml_dtypes.bfloat16)
    for t in range(NAT):
        for p in range(128):
            on[p, t * NL + 4 * t + p // 32] = 1
    in_maps = []
    for d in range(NCORES):
        xl = x[d * NL : (d + 1) * NL]                  # [NL, I, K]
        xt = xl.transpose(2, 0, 1).reshape(K, AF)      # [K, (a,i)]
        xt = np.ascontiguousarray(np.stack([xt[0:128], xt[128:256]], axis=1))
        in_maps.append({"xt": xt, "zt": zt, "ones": on})
    return in_maps


def _epilogue(results):
    S = np.empty((N, N), dtype=np.float64)
    for d in range(NCORES):
        arr = results[d]["out"].astype(np.float64)     # [64, NPAIR, 2, BJC]
        r = arr[:, :, 0, :] / arr[:, :, 1, :]          # [64, pair, 512]
        # row p = 32*q + a ; chunk c = 2*pp + q ; col = (b - 16c)*32 + j
        r = r.reshape(2, NL, NPAIR, BJC // J, J).mean(axis=4)  # [q, a, pp, 16]
        for q in range(2):
            for pp in range(NPAIR):
                c = 2 * pp + q
                S[d * NL : (d + 1) * NL, 16 * c : 16 * (c + 1)] = r[q, :, pp, :]
    diag = np.diagonal(S)
    m0 = S.max(axis=0)
    lx = m0 + np.log(np.exp(S - m0[None, :]).sum(axis=0)) - diag
    m1 = S.max(axis=1)
    lz = m1 + np.log(np.exp(S - m1[:, None]).sum(axis=1)) - diag
    loss = (lx + lz).mean()
    return np.asarray(loss, dtype=np.float32)


def run_on_device(x, z, trace=False):
    """Returns (loss, BassKernelResults)."""
    from concourse.bass_utils import run_bass_kernel_spmd

    global _cached
    if _cached is None:
        _cached = _build()
    nc = _cached
    in_maps = _prep_inputs(x, z)
    res = run_bass_kernel_spmd(nc, in_maps, list(range(NCORES)), trace=trace)
    return _epilogue(res.results), res


def kernel(x, z):
    loss, _ = run_on_device(x, z)
    return loss


# revision 24
# speedup vs baseline: 1.7469x; 1.0211x over previous
"""Bass/Trainium2 kernel for nn_GroundingLoss (symmetric token-level InfoNCE).

Math (matches the jax reference exactly):
    sim[a,b,i,j] = sum_k x[a,i,k] * z[b,j,k]
    S[a,b]       = (1/J) * sum_j  [ sum_i softmax_i(sim[a,b,:,j]) * sim[a,b,:,j] ]
    loss         = mean( logsumexp_a(S) - diag + logsumexp_b(S) - diag )

Sharding: the batch axis of x (a) is split across the 8 cores; every core
computes S[a_local, :] against all of z.

v3 design (per core):
  partitions = (a_sub, i) per a-tile (4 a's x 32 i's = 128), free = (b, j).
  sim via fp8e4m3 DoubleRow matmuls: lhsT xt8 [128,(2,128)], rhs zt8
  [128,(2,512)] -> one matmul contracts all K=256 at 0.5 cyc/col (4x fewer
  PE cycles than the bf16 K-half pair; host-measured fp8 loss rel-err
  ~1.2e-3, well inside the 2e-2 gate).  ACT computes e = exp(sim - SHIFT)
  on chunk-PAIRS [128,1024] PSUM->SBUF bf16; DVE and Pool alternate the
  es = e * sim product.  The i-reductions stay on the PE as block-diagonal
  bf16 ones-matmuls: both chunks of a pair accumulate into one PSUM bank
  via output base partitions {0, 32} (num_p/den_p [64, 512], row =
  32*(c%2) + a_local).  Each pair's num/den is DMA'd PSUM->DRAM directly.
  Loop over pair-blocks with the ones-matmuls skewed one block behind the
  sim matmuls so the PE never waits on the exp/mul chain and stays in the
  high p-state.  The host does the tiny division + j-sum + [256,256]
  logsumexp epilogue.
"""

import numpy as np

N, I, J, K = 256, 32, 32, 256
NCORES = 8
NL = N // NCORES          # 32 local a's per core
AF = NL * I               # 1024 xt cols (a, i)
BJ = N * J                # 8192 (b, j) pairs
BJC = 512                 # free elements per chunk (16 b's x 32 j's)
NCHUNK = BJ // BJC        # 16
NPAIR = NCHUNK // 2       # 8 chunk-pairs
NAT = NL // 4             # 8 a-tiles of (4 a's x 32 i's) = 128 partitions
SHIFT = 60.0              # exp shift: safe for |sim| up to ~130
SKEW = 3                  # ones-matmuls trail the sim matmuls by SKEW steps

_cached = None


def _build():
    import concourse.bacc as bacc
    import concourse.mybir as mybir
    import concourse.tile as tile

    f32 = mybir.dt.float32
    bf16 = mybir.dt.bfloat16
    fp8 = mybir.dt.float8e4
    AF_T = mybir.ActivationFunctionType
    DR = mybir.MatmulPerfMode.DoubleRow

    nc = bacc.Bacc("TRN2", target_bir_lowering=False, debug=False)
    xt_d = nc.dram_tensor("xt", [128, 2, AF], fp8, kind="ExternalInput").ap()
    zt_d = nc.dram_tensor("zt", [128, NCHUNK, 2, BJC], fp8, kind="ExternalInput").ap()
    on_d = nc.dram_tensor("ones", [128, NAT * NL], bf16, kind="ExternalInput").ap()
    # out: per pair [64, num(512) | den(512)] f32, rows = 32*(c%2) + a_local
    out_d = nc.dram_tensor("out", [64, NPAIR, 2, BJC], f32, kind="ExternalOutput").ap()

    with tile.TileContext(nc) as tc:
        with (
            tc.tile_pool(name="const", bufs=1) as cpool,
            tc.tile_pool(name="simp", bufs=3, space="PSUM") as ppool,
            tc.tile_pool(name="nd", bufs=1, space="PSUM") as ndpool,
            tc.tile_pool(name="combo", bufs=10) as copool,
            tc.tile_pool(name="ndsb", bufs=2) as ndsbpool,
        ):
            bias_t = cpool.tile([128, 1], f32)
            nc.gpsimd.memset(bias_t[:], -SHIFT)
            xt = cpool.tile([128, 2, AF], fp8)
            nc.gpsimd.dma_start(xt[:], xt_d[:, :, :])
            ones = cpool.tile([128, NAT * NL], bf16)
            nc.sync.dma_start(ones[:], on_d[:, :])
            zt = cpool.tile([128, NCHUNK, 2, BJC], fp8)
            # chunk-major z layout: each quarter is contiguous 4KB runs per
            # partition; spread across queues so chunk 0 lands early
            for qd, eng in enumerate((nc.scalar, nc.gpsimd, nc.sync, nc.scalar)):
                sl = slice(qd * 4, (qd + 1) * 4)
                eng.dma_start(zt[:, sl, :, :], zt_d[:, sl, :, :])

            # combo tile layout per (pp, t): [es(c0) es(c1) e(c0) e(c1)], bf16
            combos = {}
            nd = None
            NSTEP = NPAIR * NAT
            for u in range(NSTEP + SKEW):
                if u < NSTEP:
                    pp, t = divmod(u, NAT)
                    # sim for pair pp, atile t: [128, (2 chunks x 512)]
                    lhsT = xt[:, :, t * 128 : (t + 1) * 128]
                    sim = ppool.tile([128, 2, BJC], f32, tag="sim")
                    for h in range(2):
                        c = 2 * pp + h
                        nc.tensor.matmul(
                            sim[:, h, :], lhsT,
                            zt[:, c, :, :],
                            start=True, stop=True, perf_mode=DR,
                        )
                    co = copool.tile([128, 4, BJC], bf16, tag="combo")
                    # e pair: exp(sim - SHIFT), one ACT instr [128,1024]
                    nc.scalar.activation(
                        co[:, 2:4, :], sim[:, :, :], AF_T.Exp,
                        bias=bias_t[:], scale=1.0,
                    )
                    # es pair: e * sim, one DVE instr [128,1024]
                    nc.vector.tensor_mul(co[:, 0:2, :], co[:, 2:4, :], sim[:, :, :])
                    combos[(pp, t)] = co
                v = u - SKEW
                if v >= 0:
                    pq, tq = divmod(v, NAT)
                    # reductions for pair pq, atile tq -> stacked PSUM rows
                    onesT = ones[:, tq * NL : (tq + 1) * NL]
                    if tq == 0:
                        nd = ndpool.tile([64, 2, BJC], f32, tag="nd")
                    co = combos.pop((pq, tq))
                    st, sp = (tq == 0), (tq == NAT - 1)
                    for q in range(2):
                        nc.tensor.matmul(
                            nd[32 * q : 32 * (q + 1), 0, :],
                            onesT, co[:, q, :],
                            start=st, stop=sp,
                        )
                        nc.tensor.matmul(
                            nd[32 * q : 32 * (q + 1), 1, :],
                            onesT, co[:, 2 + q, :],
                            start=st, stop=sp,
                        )
                    if tq == NAT - 1:
                        # stage num|den to SBUF (DMA cannot read PSUM); the
                        # last block's copy runs on DVE to balance ACT/DVE
                        ndsb = ndsbpool.tile([64, 2, BJC], f32, tag="ndsb")
                        if pq == NPAIR - 1:
                            nc.vector.tensor_copy(ndsb[:], nd[:])
                        else:
                            nc.scalar.activation(ndsb[:], nd[:], AF_T.Copy)
                        nc.sync.dma_start(out_d[:, pq, :, :], ndsb[:])
    nc.compile()
    return nc


def _prep_inputs(x, z):
    import ml_dtypes

    f8 = ml_dtypes.float8_e4m3fn
    x = np.ascontiguousarray(x, dtype=np.float32).astype(f8)
    z = np.ascontiguousarray(z, dtype=np.float32).astype(f8)
    # zt[p, c, kc, col] = z[b, j, kc*128 + p] with b*J + j = c*BJC + col
    zt = z.transpose(2, 0, 1).reshape(K, BJ)
    zt = np.stack([zt[0:128], zt[128:256]], axis=1)      # [128, 2, BJ]
    zt = np.ascontiguousarray(zt.reshape(128, 2, NCHUNK, BJC).transpose(0, 2, 1, 3))
    # block-diagonal ones: tile t's lhsT [128, 32] has its 1 at column
    # 4t + p//32, so output row = a_local for the 4 a's the tile covers
    on = np.zeros((128, NAT * NL), dtype=ml_dtypes.bfloat16)
    for t in range(NAT):
        for p in range(128):
            on[p, t * NL + 4 * t + p // 32] = 1
    in_maps = []
    for d in range(NCORES):
        xl = x[d * NL : (d + 1) * NL]                  # [NL, I, K]
        xt = xl.transpose(2, 0, 1).reshape(K, AF)      # [K, (a,i)]
        xt = np.ascontiguousarray(np.stack([xt[0:128], xt[128:256]], axis=1))
        in_maps.append({"xt": xt, "zt": zt, "ones": on})
    return in_maps


def _epilogue(results):
    S = np.empty((N, N), dtype=np.float64)
    for d in range(NCORES):
        arr = results[d]["out"].astype(np.float64)     # [64, NPAIR, 2, BJC]
        r = arr[:, :, 0, :] / arr[:, :, 1, :]          # [64, pair, 512]
        # row p = 32*q + a ; chunk c = 2*pp + q ; col = (b - 16c)*32 + j
        r = r.reshape(2, NL, NPAIR, BJC // J, J).mean(axis=4)  # [q, a, pp, 16]
        for q in range(2):
            for pp in range(NPAIR):
                c = 2 * pp + q
                S[d * NL : (d + 1) * NL, 16 * c : 16 * (c + 1)] = r[q, :, pp, :]
    diag = np.diagonal(S)
    m0 = S.max(axis=0)
    lx = m0 + np.log(np.exp(S - m0[None, :]).sum(axis=0)) - diag
    m1 = S.max(axis=1)
    lz = m1 + np.log(np.exp(S - m1[:, None]).sum(axis=1)) - diag
    loss = (lx + lz).mean()
    return np.asarray(loss, dtype=np.float32)


def run_on_device(x, z, trace=False):
    """Returns (loss, BassKernelResults)."""
    from concourse.bass_utils import run_bass_kernel_spmd

    global _cached
    if _cached is None:
        _cached = _build()
    nc = _cached
    in_maps = _prep_inputs(x, z)
    res = run_bass_kernel_spmd(nc, in_maps, list(range(NCORES)), trace=trace)
    return _epilogue(res.results), res


def kernel(x, z):
    loss, _ = run_on_device(x, z)
    return loss


# revision 26
# speedup vs baseline: 1.7837x; 1.0210x over previous
"""Bass/Trainium2 kernel for nn_GroundingLoss (symmetric token-level InfoNCE).

Math (matches the jax reference exactly):
    sim[a,b,i,j] = sum_k x[a,i,k] * z[b,j,k]
    S[a,b]       = (1/J) * sum_j  [ sum_i softmax_i(sim[a,b,:,j]) * sim[a,b,:,j] ]
    loss         = mean( logsumexp_a(S) - diag + logsumexp_b(S) - diag )

Sharding: the batch axis of x (a) is split across the 8 cores; every core
computes S[a_local, :] against all of z.

v3 design (per core):
  partitions = (a_sub, i) per a-tile (4 a's x 32 i's = 128), free = (b, j).
  sim via fp8e4m3 DoubleRow matmuls: lhsT xt8 [128,(2,128)], rhs zt8
  [128,(2,512)] -> one matmul contracts all K=256 at 0.5 cyc/col (4x fewer
  PE cycles than the bf16 K-half pair; host-measured fp8 loss rel-err
  ~1.2e-3, well inside the 2e-2 gate).  ACT computes e = exp(sim - SHIFT)
  on chunk-PAIRS [128,1024] PSUM->SBUF bf16; DVE and Pool alternate the
  es = e * sim product.  The i-reductions stay on the PE as block-diagonal
  bf16 ones-matmuls: both chunks of a pair accumulate into one PSUM bank
  via output base partitions {0, 32} (num_p/den_p [64, 512], row =
  32*(c%2) + a_local).  Each pair's num/den is DMA'd PSUM->DRAM directly.
  Loop over pair-blocks with the ones-matmuls skewed one block behind the
  sim matmuls so the PE never waits on the exp/mul chain and stays in the
  high p-state.  The host does the tiny division + j-sum + [256,256]
  logsumexp epilogue.
"""

import numpy as np

N, I, J, K = 256, 32, 32, 256
NCORES = 8
NL = N // NCORES          # 32 local a's per core
AF = NL * I               # 1024 xt cols (a, i)
BJ = N * J                # 8192 (b, j) pairs
BJC = 512                 # free elements per chunk (16 b's x 32 j's)
NCHUNK = BJ // BJC        # 16
NPAIR = NCHUNK // 2       # 8 chunk-pairs
NAT = NL // 4             # 8 a-tiles of (4 a's x 32 i's) = 128 partitions
SHIFT = 60.0              # exp shift: safe for |sim| up to ~130
SKEW = 3                  # ones-matmuls trail the sim matmuls by SKEW steps

_cached = None


def _build():
    import concourse.bacc as bacc
    import concourse.mybir as mybir
    import concourse.tile as tile

    f32 = mybir.dt.float32
    bf16 = mybir.dt.bfloat16
    fp8 = mybir.dt.float8e4
    AF_T = mybir.ActivationFunctionType
    DR = mybir.MatmulPerfMode.DoubleRow

    nc = bacc.Bacc("TRN2", target_bir_lowering=False, debug=False)
    xt_d = nc.dram_tensor("xt", [128, 2, AF], fp8, kind="ExternalInput").ap()
    zt_d = nc.dram_tensor("zt", [128, NCHUNK, 2, BJC], fp8, kind="ExternalInput").ap()
    on_d = nc.dram_tensor("ones", [128, NAT * NL], bf16, kind="ExternalInput").ap()
    # out: per pair [64, num(512) | den(512)] f32, rows = 32*(c%2) + a_local
    out_d = nc.dram_tensor("out", [64, NPAIR, 2, BJC], f32, kind="ExternalOutput").ap()

    with tile.TileContext(nc) as tc:
        with (
            tc.tile_pool(name="const", bufs=1) as cpool,
            tc.tile_pool(name="simp", bufs=3, space="PSUM") as ppool,
            tc.tile_pool(name="nd", bufs=1, space="PSUM") as ndpool,
            tc.tile_pool(name="combo", bufs=10) as copool,
            tc.tile_pool(name="ndsb", bufs=2) as ndsbpool,
        ):
            bias_t = cpool.tile([128, 1], f32)
            nc.gpsimd.memset(bias_t[:], -SHIFT)
            xt = cpool.tile([128, 2, AF], fp8)
            nc.gpsimd.dma_start(xt[:], xt_d[:, :, :])
            ones = cpool.tile([128, NAT * NL], bf16)
            nc.sync.dma_start(ones[:], on_d[:, :])
            zt = cpool.tile([128, NCHUNK, 2, BJC], fp8)
            # chunk-major z layout: contiguous 4KB runs per partition.  Only
            # the first half is fetched upfront (so HBM bandwidth goes to the
            # chunks the warmup needs); the rest is issued inside the loop.
            for qd, eng in ((0, nc.scalar), (1, nc.sync)):
                sl = slice(qd * 4, (qd + 1) * 4)
                eng.dma_start(zt[:, sl, :, :], zt_d[:, sl, :, :])

            # PE warmup: dummy matmuls on memset scratch ramp the tensor
            # engine's p-state while the input DMAs are still in flight
            scr = cpool.tile([128, BJC], bf16)
            nc.gpsimd.memset(scr[:], 0.0)
            warm = ppool.tile([128, 2, BJC], f32, tag="sim")
            for w in range(12):
                nc.tensor.matmul(
                    warm[0:32, 0, :], scr[:, 0:32], scr[:, :],
                    start=True, stop=True,
                )

            # combo tile layout per (pp, t): [es(c0) es(c1) e(c0) e(c1)], bf16
            combos = {}
            nd = None
            NSTEP = NPAIR * NAT
            for u in range(NSTEP + SKEW):
                if u == 2:
                    nc.scalar.dma_start(zt[:, 8:12, :, :], zt_d[:, 8:12, :, :])
                if u == 6:
                    nc.sync.dma_start(zt[:, 12:16, :, :], zt_d[:, 12:16, :, :])
                if u < NSTEP:
                    pp, t = divmod(u, NAT)
                    # sim for pair pp, atile t: [128, (2 chunks x 512)]
                    lhsT = xt[:, :, t * 128 : (t + 1) * 128]
                    sim = ppool.tile([128, 2, BJC], f32, tag="sim")
                    for h in range(2):
                        c = 2 * pp + h
                        nc.tensor.matmul(
                            sim[:, h, :], lhsT,
                            zt[:, c, :, :],
                            start=True, stop=True, perf_mode=DR,
                        )
                    co = copool.tile([128, 4, BJC], bf16, tag="combo")
                    # e pair: exp(sim - SHIFT), one ACT instr [128,1024]
                    nc.scalar.activation(
                        co[:, 2:4, :], sim[:, :, :], AF_T.Exp,
                        bias=bias_t[:], scale=1.0,
                    )
                    # es pair: e * sim, one DVE instr [128,1024]
                    nc.vector.tensor_mul(co[:, 0:2, :], co[:, 2:4, :], sim[:, :, :])
                    combos[(pp, t)] = co
                v = u - SKEW
                if v >= 0:
                    pq, tq = divmod(v, NAT)
                    # reductions for pair pq, atile tq -> stacked PSUM rows
                    onesT = ones[:, tq * NL : (tq + 1) * NL]
                    if tq == 0:
                        nd = ndpool.tile([64, 2, BJC], f32, tag="nd")
                    co = combos.pop((pq, tq))
                    st, sp = (tq == 0), (tq == NAT - 1)
                    for q in range(2):
                        nc.tensor.matmul(
                            nd[32 * q : 32 * (q + 1), 0, :],
                            onesT, co[:, q, :],
                            start=st, stop=sp,
                        )
                        nc.tensor.matmul(
                            nd[32 * q : 32 * (q + 1), 1, :],
                            onesT, co[:, 2 + q, :],
                            start=st, stop=sp,
                        )
                    if tq == NAT - 1:
                        # stage num|den to SBUF (DMA cannot read PSUM); the
                        # last block's copy runs on DVE to balance ACT/DVE
                        ndsb = ndsbpool.tile([64, 2, BJC], f32, tag="ndsb")
                        if pq == NPAIR - 1:
                            nc.vector.tensor_copy(ndsb[:], nd[:])
                        else:
                            nc.scalar.activation(ndsb[:], nd[:], AF_T.Copy)
                        nc.sync.dma_start(out_d[:, pq, :, :], ndsb[:])
    nc.compile()
    return nc


def _prep_inputs(x, z):
    import ml_dtypes

    f8 = ml_dtypes.float8_e4m3fn
    x = np.ascontiguousarray(x, dtype=np.float32).astype(f8)
    z = np.ascontiguousarray(z, dtype=np.float32).astype(f8)
    # zt[p, c, kc, col] = z[b, j, kc*128 + p] with b*J + j = c*BJC + col
    zt = z.transpose(2, 0, 1).reshape(K, BJ)
    zt = np.stack([zt[0:128], zt[128:256]], axis=1)      # [128, 2, BJ]
    zt = np.ascontiguousarray(zt.reshape(128, 2, NCHUNK, BJC).transpose(0, 2, 1, 3))
    # block-diagonal ones: tile t's lhsT [128, 32] has its 1 at column
    # 4t + p//32, so output row = a_local for the 4 a's the tile covers
    on = np.zeros((128, NAT * NL), dtype=ml_dtypes.bfloat16)
    for t in range(NAT):
        for p in range(128):
            on[p, t * NL + 4 * t + p // 32] = 1
    in_maps = []
    for d in range(NCORES):
        xl = x[d * NL : (d + 1) * NL]                  # [NL, I, K]
        xt = xl.transpose(2, 0, 1).reshape(K, AF)      # [K, (a,i)]
        xt = np.ascontiguousarray(np.stack([xt[0:128], xt[128:256]], axis=1))
        in_maps.append({"xt": xt, "zt": zt, "ones": on})
    return in_maps


def _epilogue(results):
    S = np.empty((N, N), dtype=np.float64)
    for d in range(NCORES):
        arr = results[d]["out"].astype(np.float64)     # [64, NPAIR, 2, BJC]
        r = arr[:, :, 0, :] / arr[:, :, 1, :]          # [64, pair, 512]
        # row p = 32*q + a ; chunk c = 2*pp + q ; col = (b - 16c)*32 + j
        r = r.reshape(2, NL, NPAIR, BJC // J, J).mean(axis=4)  # [q, a, pp, 16]
        for q in range(2):
            for pp in range(NPAIR):
                c = 2 * pp + q
                S[d * NL : (d + 1) * NL, 16 * c : 16 * (c + 1)] = r[q, :, pp, :]
    diag = np.diagonal(S)
    m0 = S.max(axis=0)
    lx = m0 + np.log(np.exp(S - m0[None, :]).sum(axis=0)) - diag
    m1 = S.max(axis=1)
    lz = m1 + np.log(np.exp(S - m1[:, None]).sum(axis=1)) - diag
    loss = (lx + lz).mean()
    return np.asarray(loss, dtype=np.float32)


def run_on_device(x, z, trace=False):
    """Returns (loss, BassKernelResults)."""
    from concourse.bass_utils import run_bass_kernel_spmd

    global _cached
    if _cached is None:
        _cached = _build()
    nc = _cached
    in_maps = _prep_inputs(x, z)
    res = run_bass_kernel_spmd(nc, in_maps, list(range(NCORES)), trace=trace)
    return _epilogue(res.results), res


def kernel(x, z):
    loss, _ = run_on_device(x, z)
    return loss


# revision 31
# speedup vs baseline: 1.7885x; 1.0027x over previous
"""Bass/Trainium2 kernel for nn_GroundingLoss (symmetric token-level InfoNCE).

Math (matches the jax reference exactly):
    sim[a,b,i,j] = sum_k x[a,i,k] * z[b,j,k]
    S[a,b]       = (1/J) * sum_j  [ sum_i softmax_i(sim[a,b,:,j]) * sim[a,b,:,j] ]
    loss         = mean( logsumexp_a(S) - diag + logsumexp_b(S) - diag )

Sharding: the batch axis of x (a) is split across the 8 cores; every core
computes S[a_local, :] against all of z.

v3 design (per core):
  partitions = (a_sub, i) per a-tile (4 a's x 32 i's = 128), free = (b, j).
  sim via fp8e4m3 DoubleRow matmuls: lhsT xt8 [128,(2,128)], rhs zt8
  [128,(2,512)] -> one matmul contracts all K=256 at 0.5 cyc/col (4x fewer
  PE cycles than the bf16 K-half pair; host-measured fp8 loss rel-err
  ~1.2e-3, well inside the 2e-2 gate).  ACT computes e = exp(sim - SHIFT)
  on chunk-PAIRS [128,1024] PSUM->SBUF bf16; DVE and Pool alternate the
  es = e * sim product.  The i-reductions stay on the PE as block-diagonal
  bf16 ones-matmuls: both chunks of a pair accumulate into one PSUM bank
  via output base partitions {0, 32} (num_p/den_p [64, 512], row =
  32*(c%2) + a_local).  Each pair's num/den is DMA'd PSUM->DRAM directly.
  Loop over pair-blocks with the ones-matmuls skewed one block behind the
  sim matmuls so the PE never waits on the exp/mul chain and stays in the
  high p-state.  The host does the tiny division + j-sum + [256,256]
  logsumexp epilogue.
"""

import numpy as np

N, I, J, K = 256, 32, 32, 256
NCORES = 8
NL = N // NCORES          # 32 local a's per core
AF = NL * I               # 1024 xt cols (a, i)
BJ = N * J                # 8192 (b, j) pairs
BJC = 512                 # free elements per chunk (16 b's x 32 j's)
NCHUNK = BJ // BJC        # 16
NPAIR = NCHUNK // 2       # 8 chunk-pairs
NAT = NL // 4             # 8 a-tiles of (4 a's x 32 i's) = 128 partitions
SHIFT = 60.0              # exp shift: safe for |sim| up to ~130
SKEW = 3                  # ones-matmuls trail the sim matmuls by SKEW steps

_cached = None


def _build():
    import concourse.bacc as bacc
    import concourse.mybir as mybir
    import concourse.tile as tile

    f32 = mybir.dt.float32
    bf16 = mybir.dt.bfloat16
    fp8 = mybir.dt.float8e4
    AF_T = mybir.ActivationFunctionType
    DR = mybir.MatmulPerfMode.DoubleRow

    nc = bacc.Bacc("TRN2", target_bir_lowering=False, debug=False)
    xt_d = nc.dram_tensor("xt", [128, 2, AF], fp8, kind="ExternalInput").ap()
    zt_d = nc.dram_tensor("zt", [128, NCHUNK, 2, BJC], fp8, kind="ExternalInput").ap()
    on_d = nc.dram_tensor("ones", [128, NAT * NL], bf16, kind="ExternalInput").ap()
    # out: per pair [64, num(512) | den(512)] f32, rows = 32*(c%2) + a_local
    out_d = nc.dram_tensor("out", [64, NPAIR, 2, BJC], f32, kind="ExternalOutput").ap()

    with tile.TileContext(nc) as tc:
        with (
            tc.tile_pool(name="const", bufs=1) as cpool,
            tc.tile_pool(name="simp", bufs=3, space="PSUM") as ppool,
            tc.tile_pool(name="nd", bufs=1, space="PSUM") as ndpool,
            tc.tile_pool(name="combo", bufs=10) as copool,
            tc.tile_pool(name="ndsb", bufs=2) as ndsbpool,
        ):
            bias_t = cpool.tile([128, 1], f32)
            nc.gpsimd.memset(bias_t[:], -SHIFT)
            xt = cpool.tile([128, 2, AF], fp8)
            nc.gpsimd.dma_start(xt[:], xt_d[:, :, :])
            ones = cpool.tile([128, NAT * NL], bf16)
            nc.sync.dma_start(ones[:], on_d[:, :])
            zt = cpool.tile([128, NCHUNK, 2, BJC], fp8)
            # chunk-major z layout: contiguous 4KB runs per partition.  Only
            # the first half is fetched upfront (so HBM bandwidth goes to the
            # chunks the warmup needs); the rest is issued inside the loop.
            nc.scalar.dma_start(zt[:, 0:4, :, :], zt_d[:, 0:4, :, :])

            # PE warmup: dummy matmuls on memset scratch ramp the tensor
            # engine's p-state while the input DMAs are still in flight
            scr = cpool.tile([128, BJC], bf16)
            nc.gpsimd.memset(scr[:], 0.0)
            warm = ppool.tile([128, 2, BJC], f32, tag="sim")
            for w in range(10):
                nc.tensor.matmul(
                    warm[0:32, 0, :], scr[:, 0:32], scr[:, :],
                    start=True, stop=True,
                )

            # combo tile layout per (pp, t): [es(c0) es(c1) e(c0) e(c1)], bf16
            combos = {}
            nd = None
            NSTEP = NPAIR * NAT
            for u in range(NSTEP + SKEW):
                if u == 0:
                    nc.sync.dma_start(zt[:, 4:8, :, :], zt_d[:, 4:8, :, :])
                if u == 4:
                    nc.scalar.dma_start(zt[:, 8:12, :, :], zt_d[:, 8:12, :, :])
                if u == 8:
                    nc.sync.dma_start(zt[:, 12:16, :, :], zt_d[:, 12:16, :, :])
                if u < NSTEP:
                    pp, t = divmod(u, NAT)
                    # sim for pair pp, atile t: [128, (2 chunks x 512)]
                    lhsT = xt[:, :, t * 128 : (t + 1) * 128]
                    sim = ppool.tile([128, 2, BJC], f32, tag="sim")
                    for h in range(2):
                        c = 2 * pp + h
                        nc.tensor.matmul(
                            sim[:, h, :], lhsT,
                            zt[:, c, :, :],
                            start=True, stop=True, perf_mode=DR,
                        )
                    co = copool.tile([128, 4, BJC], bf16, tag="combo")
                    # e pair: exp(sim - SHIFT), one ACT instr [128,1024]
                    nc.scalar.activation(
                        co[:, 2:4, :], sim[:, :, :], AF_T.Exp,
                        bias=bias_t[:], scale=1.0,
                    )
                    # es pair: e * sim, one DVE instr [128,1024]
                    nc.vector.tensor_mul(co[:, 0:2, :], co[:, 2:4, :], sim[:, :, :])
                    combos[(pp, t)] = co
                v = u - SKEW
                if v >= 0:
                    pq, tq = divmod(v, NAT)
                    # reductions for pair pq, atile tq -> stacked PSUM rows
                    onesT = ones[:, tq * NL : (tq + 1) * NL]
                    if tq == 0:
                        nd = ndpool.tile([64, 2, BJC], f32, tag="nd")
                    co = combos.pop((pq, tq))
                    st, sp = (tq == 0), (tq == NAT - 1)
                    for q in range(2):
                        nc.tensor.matmul(
                            nd[32 * q : 32 * (q + 1), 0, :],
                            onesT, co[:, q, :],
                            start=st, stop=sp,
                        )
                        nc.tensor.matmul(
                            nd[32 * q : 32 * (q + 1), 1, :],
                            onesT, co[:, 2 + q, :],
                            start=st, stop=sp,
                        )
                    if tq == NAT - 1:
                        # stage num|den to SBUF (DMA cannot read PSUM); the
                        # last block's copy runs on DVE to balance ACT/DVE
                        ndsb = ndsbpool.tile([64, 2, BJC], f32, tag="ndsb")
                        if pq == NPAIR - 1:
                            nc.vector.tensor_copy(ndsb[:], nd[:])
                        else:
                            nc.scalar.activation(ndsb[:], nd[:], AF_T.Copy)
                        nc.sync.dma_start(out_d[:, pq, :, :], ndsb[:])
    nc.compile()
    return nc


def _prep_inputs(x, z):
    import ml_dtypes

    f8 = ml_dtypes.float8_e4m3fn
    x = np.ascontiguousarray(x, dtype=np.float32).astype(f8)
    z = np.ascontiguousarray(z, dtype=np.float32).astype(f8)
    # zt[p, c, kc, col] = z[b, j, kc*128 + p] with b*J + j = c*BJC + col
    zt = z.transpose(2, 0, 1).reshape(K, BJ)
    zt = np.stack([zt[0:128], zt[128:256]], axis=1)      # [128, 2, BJ]
    zt = np.ascontiguousarray(zt.reshape(128, 2, NCHUNK, BJC).transpose(0, 2, 1, 3))
    # block-diagonal ones: tile t's lhsT [128, 32] has its 1 at column
    # 4t + p//32, so output row = a_local for the 4 a's the tile covers
    on = np.zeros((128, NAT * NL), dtype=ml_dtypes.bfloat16)
    for t in range(NAT):
        for p in range(128):
            on[p, t * NL + 4 * t + p // 32] = 1
    in_maps = []
    for d in range(NCORES):
        xl = x[d * NL : (d + 1) * NL]                  # [NL, I, K]
        xt = xl.transpose(2, 0, 1).reshape(K, AF)      # [K, (a,i)]
        xt = np.ascontiguousarray(np.stack([xt[0:128], xt[128:256]], axis=1))
        in_maps.append({"xt": xt, "zt": zt, "ones": on})
    return in_maps


def _epilogue(results):
    S = np.empty((N, N), dtype=np.float64)
    for d in range(NCORES):
        arr = results[d]["out"].astype(np.float64)     # [64, NPAIR, 2, BJC]
        r = arr[:, :, 0, :] / arr[:, :, 1, :]          # [64, pair, 512]
        # row p = 32*q + a ; chunk c = 2*pp + q ; col = (b - 16c)*32 + j
        r = r.reshape(2, NL, NPAIR, BJC // J, J).mean(axis=4)  # [q, a, pp, 16]
        for q in range(2):
            for pp in range(NPAIR):
                c = 2 * pp + q
                S[d * NL : (d + 1) * NL, 16 * c : 16 * (c + 1)] = r[q, :, pp, :]
    diag = np.diagonal(S)
    m0 = S.max(axis=0)
    lx = m0 + np.log(np.exp(S - m0[None, :]).sum(axis=0)) - diag
    m1 = S.max(axis=1)
    lz = m1 + np.log(np.exp(S - m1[:, None]).sum(axis=1)) - diag
    loss = (lx + lz).mean()
    return np.asarray(loss, dtype=np.float32)


def run_on_device(x, z, trace=False):
    """Returns (loss, BassKernelResults)."""
    from concourse.bass_utils import run_bass_kernel_spmd

    global _cached
    if _cached is None:
        _cached = _build()
    nc = _cached
    in_maps = _prep_inputs(x, z)
    res = run_bass_kernel_spmd(nc, in_maps, list(range(NCORES)), trace=trace)
    return _epilogue(res.results), res


def kernel(x, z):
    loss, _ = run_on_device(x, z)
    return loss
